# revision 1
# baseline (speedup 1.0000x reference)
"""CQAttention (trilinear context-query attention) Bass kernel for TRN2.

Full-input contract: kernel(**inputs) takes the unsharded tensors
  C (1024, 64, 256), Q (512, 64, 256), w4C (256,1), w4Q (256,1),
  w4mlu (1,1,256), bias (1,)
and returns out (64, 1024, 1024) fp32, matching the reference

  C,Q -> batch-major; S = C@w4C + (Q@w4Q)^T + (C*w4mlu)@Q^T + bias
  S1 = softmax_q(S); S2 = softmax_c(S)
  A = S1@Q ; B = (S1@S2^T)@C
  out = concat([C, A, C*A, C*B], -1) transposed to (B, 4D, Lc)

Sharding: data-parallel over batch, 8 batch items per NeuronCore.

Algebra used on-chip (per batch item):
  * bias cancels in both softmaxes (constant shift) -> dropped.
  * e0 = exp(C@w4C), e1 = exp(Q@w4Q), E0 = exp((C*w4mlu)@Q^T) so that
    exp(S) = e0[c] * E0[c,q] * e1[q].
  * S1 = diag(1/rs) E0 diag(e1),  rs  = E0 @ e1          (e0 cancels)
  * S2 = diag(e0) E0 diag(1/cs),  cs  = E0^T @ e0        (e1 cancels)
  * A    = diag(1/rs) (E0 @ (diag(e1) Q))
  * S2^T C = diag(1/cs) (E0^T @ (diag(e0) C))
  * B    = S1 @ (S2^T C) = diag(1/rs) (E0 @ (diag(e1/cs) (E0^T (diag(e0) C))))
  * (S1@S2^T)@C reassociated as S1@(S2^T@C): halves the matmul FLOPs.
  Everything is computed transposed ([feature, context] layout) so output
  DMA rows are contiguous in DRAM.
"""

import numpy as np

LC, LQ, B, D = 1024, 512, 64, 256
NCORES = 8
BPC = B // NCORES  # batch items per core
P = 128
MC = LC // P  # 8 context chunks
TQ = LQ // P  # 4 query chunks
KD = D // P   # 2 feature chunks

# float32r: single-pass relaxed-precision fp32 matmul (1 cyc/row at N>=256)
# float32:  exact two-pass fp32 matmul (4 cyc/row)
MM_RELAXED = True

_CACHE = {}


def _ensure_path():
    import sys
    for p in ("/opt/trn_rl_repo",):
        if p not in sys.path:
            sys.path.insert(0, p)


def _build_nc(mm_relaxed=MM_RELAXED):
    _ensure_path()
    import concourse.bass as bass
    import concourse.bacc as bacc
    import concourse.mybir as mybir
    from concourse import tile, masks

    f32 = mybir.dt.float32
    mmdt = mybir.dt.float32r if mm_relaxed else f32
    Exp = mybir.ActivationFunctionType.Exp
    Copy = mybir.ActivationFunctionType.Copy
    mult = mybir.AluOpType.mult
    AxX = mybir.AxisListType.X
    add = mybir.AluOpType.add

    def r(ap):
        return ap.bitcast(mmdt)

    nc = bacc.Bacc()
    C_d = nc.dram_tensor("C", [LC, BPC, D], f32, kind="ExternalInput")
    Q_d = nc.dram_tensor("Q", [LQ, BPC, D], f32, kind="ExternalInput")
    w4C_d = nc.dram_tensor("w4C", [D, 1], f32, kind="ExternalInput")
    w4Q_d = nc.dram_tensor("w4Q", [D, 1], f32, kind="ExternalInput")
    w4mlu_d = nc.dram_tensor("w4mlu", [1, 1, D], f32, kind="ExternalInput")
    out_d = nc.dram_tensor("out", [BPC, 4 * D, LC], f32, kind="ExternalOutput")

    with tile.TileContext(nc) as tc:
        import contextlib

        with contextlib.ExitStack() as ctx:
            ep = ctx.enter_context

            consts = ep(tc.tile_pool(name="consts", bufs=1))
            import os as _os0
            cn_pool = ep(tc.tile_pool(name="cn", bufs=int(_os0.environ.get("K_CN","2"))))
            qn_pool = ep(tc.tile_pool(name="qn", bufs=int(_os0.environ.get("K_CN","2"))))
            ct_pool = ep(tc.tile_pool(name="ct", bufs=2))
            ctr_pool = ep(tc.tile_pool(name="ctr", bufs=1))
            qt_pool = ep(tc.tile_pool(name="qt", bufs=int(_os0.environ.get("K_QT","1"))))
            qmt_pool = ep(tc.tile_pool(name="qmt", bufs=int(_os0.environ.get("K_QT","1"))))
            ce_pool = ep(tc.tile_pool(name="ce", bufs=2))
            qe_pool = ep(tc.tile_pool(name="qe", bufs=2))
            e0_pool = ep(tc.tile_pool(name="e0p", bufs=2))
            e0t_pool = ep(tc.tile_pool(name="e0tp", bufs=2))
            h2_pool = ep(tc.tile_pool(name="h2", bufs=2))
            rsbr_pool = ep(tc.tile_pool(name="rsbr", bufs=1))
            at_pool = ep(tc.tile_pool(name="at", bufs=int(_os0.environ.get("K_AT","2"))))
            bt_pool = ep(tc.tile_pool(name="bt", bufs=int(_os0.environ.get("K_AT","2"))))
            # O2 reuses ce_pool slots (Ce dead after P2); O3 reuses e0t slots
            o2_pool = ce_pool
            o3_pool = e0t_pool
            small_pool = ep(tc.tile_pool(name="small", bufs=4))
            scr_pool = ep(tc.tile_pool(name="scr", bufs=1))
            row_pool = ep(tc.tile_pool(name="rows", bufs=1))

            import os as _os2
            _psa = int(_os2.environ.get("K_PSA", "4"))
            _psrow = int(_os2.environ.get("K_PSROW", "2"))
            psA = ep(tc.tile_pool(name="psA", bufs=_psa, space="PSUM"))
            psB = ep(tc.tile_pool(name="psB", bufs=int(_os2.environ.get("K_PSB","2")), space="PSUM"))
            psRow = ep(tc.tile_pool(name="psRow", bufs=_psrow, space="PSUM"))

            # ---- per-core constants ----
            ident = consts.tile([P, P], f32)
            masks.make_identity(nc, ident[:])
            ones_row = consts.tile([1, P], f32)
            nc.vector.memset(ones_row[:], 1.0)
            ones_r = consts.tile([1, P], f32)
            nc.scalar.copy(r(ones_r[:]), ones_row[:])
            w4mlu_pp = consts.tile([P, KD], f32)
            nc.sync.dma_start(
                w4mlu_pp[:], w4mlu_d[0, 0, :].rearrange("(k p) -> p k", p=P)
            )
            # matvec weights replicated across partitions via broadcast DMA
            w4Cb = consts.tile([P, D], f32)
            nc.sync.dma_start(
                w4Cb[:],
                w4C_d[:, 0].rearrange("(a d) -> a d", a=1).broadcast_to([P, D]),
            )
            w4Qb = consts.tile([P, D], f32)
            nc.sync.dma_start(
                w4Qb[:],
                w4Q_d[:, 0].rearrange("(a d) -> a d", a=1).broadcast_to([P, D]),
            )

            import os as _os
            _nb = int(_os.environ.get("K_EMIT_BATCHES", str(BPC)))
            _ph = int(_os.environ.get("K_EMIT_PHASE", "99"))
            class _ActShim:
                def tensor_copy(self, out, in_):
                    return nc.scalar.copy(out, in_)
                def tensor_scalar_mul(self, out, in_, s):
                    return nc.scalar.activation(out, in_, Copy, scale=s)
            _act_shim = _ActShim()
            _ect = nc.vector if _os.environ.get("K_ECT", "act") == "dve" else _act_shim
            _eh2 = nc.vector if _os.environ.get("K_EH2", "act") == "dve" else _act_shim
            _pro_state = {}

            def _prologue(b):
                # loads (natural layouts) + gpsimd matvec mults for batch b
                Cn = cn_pool.tile([P, MC * D], f32, tag="cn")
                for m in range(MC):
                    nc.sync.dma_start(
                        Cn[:, m * D:(m + 1) * D], C_d[m * P:(m + 1) * P, b, :]
                    )
                Qn = qn_pool.tile([P, TQ * D], f32, tag="qn")
                for t in range(TQ):
                    nc.sync.dma_start(
                        Qn[:, t * D:(t + 1) * D], Q_d[t * P:(t + 1) * P, b, :]
                    )
                scr = scr_pool.tile([P, MC * D], f32, tag="scrA")
                w4Cb_bc = w4Cb[:].rearrange("p (a d) -> p a d", a=1) \
                    .broadcast_to([P, MC // 2, D])
                for h in range(2):
                    hs = slice(h * (MC // 2) * D, (h + 1) * (MC // 2) * D)
                    nc.gpsimd.tensor_tensor(
                        scr[:, hs].rearrange("p (m d) -> p m d", m=MC // 2),
                        Cn[:, hs].rearrange("p (m d) -> p m d", m=MC // 2),
                        w4Cb_bc, mult,
                    )
                scr1 = scr_pool.tile([P, MC * D], f32, tag="scrA")
                w4Qb_bc = w4Qb[:].rearrange("p (a d) -> p a d", a=1) \
                    .broadcast_to([P, TQ, D])
                nc.gpsimd.tensor_tensor(
                    scr1[:, 0:TQ * D].rearrange("p (t d) -> p t d", t=TQ),
                    Qn[:].rearrange("p (t d) -> p t d", t=TQ),
                    w4Qb_bc, mult,
                )
                _pro_state[b] = (Cn, Qn, scr, scr1)

            _prologue(0)
            for b in range(_nb):
                Cn, Qn, scr, scr1 = _pro_state.pop(b)
                if b + 1 < _nb:
                    _prologue(b + 1)
                # ---- transposes: CT [d,(k,c)], QT [d,(k,q)] ----
                CT = ct_pool.tile([P, KD * LC], f32)
                CTr = ctr_pool.tile([P, KD * LC], f32)
                for k in range(KD):
                    for mg in range(0, MC, 4):
                        pst = psA.tile([P, 4 * P], f32, tag="psA")
                        for j in range(4):
                            m = mg + j
                            nc.tensor.transpose(
                                pst[:, j * P:(j + 1) * P],
                                Cn[:, m * D + k * P: m * D + (k + 1) * P],
                                ident[:],
                            )
                        _ect.tensor_copy(
                            CT[:, k * LC + mg * P: k * LC + (mg + 4) * P], pst[:]
                        )
                        nc.vector.tensor_copy(
                            r(CTr[:, k * LC + mg * P: k * LC + (mg + 4) * P]),
                            pst[:],
                        )
                QT = qt_pool.tile([P, KD * LQ], f32)
                for k in range(KD):
                    pst = psA.tile([P, 4 * P], f32, tag="psA")
                    for t in range(TQ):
                        nc.tensor.transpose(
                            pst[:, t * P:(t + 1) * P],
                            Qn[:, t * D + k * P: t * D + (k + 1) * P],
                            ident[:],
                        )
                    _ect.tensor_copy(QT[:, k * LQ: k * LQ + 4 * P], pst[:])

                # matvec reduces + exps (after evacs to keep ACT/DVE queues clear)
                sub0 = small_pool.tile([P, MC], f32)
                for h in range(2):
                    hs = slice(h * (MC // 2) * D, (h + 1) * (MC // 2) * D)
                    nc.vector.tensor_reduce(
                        sub0[:, h * (MC // 2):(h + 1) * (MC // 2)],
                        scr[:, hs].rearrange("p (m d) -> p m d", m=MC // 2),
                        axis=AxX, op=add,
                    )
                e0 = small_pool.tile([P, MC], f32)
                nc.scalar.activation(r(e0[:]), sub0[:], Exp)
                sub1 = small_pool.tile([P, TQ], f32)
                nc.vector.tensor_reduce(
                    sub1[:], scr1[:, 0:TQ * D].rearrange("p (t d) -> p t d", t=TQ),
                    axis=AxX, op=add,
                )
                e1 = small_pool.tile([P, TQ], f32)
                nc.scalar.activation(r(e1[:]), sub1[:], Exp)
                for k in range(KD):
                    nc.sync.dma_start(
                        out_d[b, k * P:(k + 1) * P, :], CT[:, k * LC:(k + 1) * LC]
                    )
                # QmT = QT * w4mlu (per-partition over d)
                QmT = qmt_pool.tile([P, KD * LQ], f32)
                for k in range(KD):
                    nc.vector.tensor_scalar_mul(
                        r(QmT[:, k * LQ:(k + 1) * LQ]),
                        QT[:, k * LQ:(k + 1) * LQ],
                        w4mlu_pp[:, k:k + 1],
                    )

                # Ce = C * e0, Qe = Q * e1 (per-partition scales)
                DA = D + 2
                Ce = ce_pool.tile([P, MC * DA], f32, tag="ceA")
                for m in range(MC):
                    nc.vector.tensor_scalar_mul(
                        r(Ce[:, m * DA:m * DA + D]), Cn[:, m * D:(m + 1) * D],
                        e0[:, m:m + 1],
                    )
                    nc.vector.tensor_copy(
                        r(Ce[:, m * DA + D:m * DA + DA]),
                        e0[:, m:m + 1].broadcast_to([P, 2]),
                    )
                Qe = qe_pool.tile([P, TQ * D], f32)
                for t in range(TQ):
                    nc.vector.tensor_scalar_mul(
                        r(Qe[:, t * D:(t + 1) * D]), Qn[:, t * D:(t + 1) * D],
                        e1[:, t:t + 1],
                    )

                # ---- E0 = exp((C*w)@Q^T) [c,(m,q)] ----
                E0 = e0_pool.tile([P, MC * LQ], f32)
                for m in range(MC):
                    ps = psA.tile([P, LQ], f32, tag="psA")
                    for k in range(KD):
                        nc.tensor.matmul(
                            ps[:],
                            r(CTr[:, k * LC + m * P: k * LC + (m + 1) * P]),
                            r(QmT[:, k * LQ:(k + 1) * LQ]),
                            start=(k == 0),
                            stop=(k == KD - 1),
                        )
                    nc.scalar.activation(r(E0[:, m * LQ:(m + 1) * LQ]), ps[:], Exp)

                # ---- E0T = exp(transposed scores) [q,(t,c)] ----
                E0T = e0t_pool.tile([P, TQ * LC], f32, tag="e0tA")
                for t in range(TQ):
                    for n in range(2):
                        ps = psA.tile([P, 512], f32, tag="psA")
                        for k in range(KD):
                            nc.tensor.matmul(
                                ps[:],
                                r(QmT[:, k * LQ + t * P: k * LQ + (t + 1) * P]),
                                r(CTr[:, k * LC + n * 512: k * LC + (n + 1) * 512]),
                                start=(k == 0),
                                stop=(k == KD - 1),
                            )
                        nc.scalar.activation(
                            r(E0T[:, t * LC + n * 512: t * LC + (n + 1) * 512]),
                            ps[:], Exp,
                        )

                # ---- rs = E0 @ e1 as a row; rsbr = 1/rs replicated ----
                rs_row = row_pool.tile([1, LC], f32, tag="rowA")
                for n in range(2):
                    psr = psRow.tile([1, 512], f32)
                    for t in range(TQ):
                        nc.tensor.matmul(
                            psr[:],
                            r(e1[:, t:t + 1]),
                            r(E0T[:, t * LC + n * 512: t * LC + (n + 1) * 512]),
                            start=(t == 0),
                            stop=(t == TQ - 1),
                        )
                    nc.scalar.copy(r(rs_row[:, n * 512:(n + 1) * 512]), psr[:])
                rsbr = rsbr_pool.tile([P, LC], f32, tag="rsbr")
                for n in range(2):
                    ps = psB.tile([P, 512], f32, tag="psB")
                    nc.tensor.matmul(
                        ps[:], r(ones_r[:]), r(rs_row[:, n * 512:(n + 1) * 512])
                    )
                    nc.vector.reciprocal(rsbr[:, n * 512:(n + 1) * 512], ps[:])

                rec_cse = small_pool.tile([P, TQ], f32)
                dq = small_pool.tile([P, TQ], f32)

                # ---- P2 = E0^T @ Ce ; H2 = dq * P2  [q,(t,d)] ----
                H2 = h2_pool.tile([P, TQ * D], f32)
                for qm in range(TQ):
                    ps = psB.tile([P, 512], f32, tag="psB")
                    for m in range(MC):
                        nc.tensor.matmul(
                            ps[:, 0:DA],
                            r(E0[:, m * LQ + qm * P: m * LQ + (qm + 1) * P]),
                            r(Ce[:, m * DA:(m + 1) * DA]),
                            start=(m == 0),
                            stop=(m == MC - 1),
                        )
                    nc.vector.reciprocal(rec_cse[:, qm:qm + 1], ps[:, D:D + 1])
                    nc.vector.tensor_tensor(
                        dq[:, qm:qm + 1], rec_cse[:, qm:qm + 1], e1[:, qm:qm + 1],
                        mult,
                    )
                    _eh2.tensor_scalar_mul(
                        r(H2[:, qm * D:(qm + 1) * D]), ps[:, 0:D],
                        dq[:, qm:qm + 1],
                    )

                # ---- P1T = Qe^T @ E0T -> AT ; O2 = CT*AT ----
                AT = at_pool.tile([P, KD * LC], f32)
                O2 = o2_pool.tile([P, KD * LC], f32, tag="ceA")
                for m2 in range(KD):
                    for n in range(2):
                        ps = psB.tile([P, 512], f32, tag="psB")
                        for t in range(TQ):
                            nc.tensor.matmul(
                                ps[:],
                                r(Qe[:, t * D + m2 * P: t * D + (m2 + 1) * P]),
                                r(E0T[:, t * LC + n * 512: t * LC + (n + 1) * 512]),
                                start=(t == 0),
                                stop=(t == TQ - 1),
                            )
                        sl = slice(m2 * LC + n * 512, m2 * LC + (n + 1) * 512)
                        nsl = slice(n * 512, (n + 1) * 512)
                        nc.vector.tensor_tensor(AT[:, sl], ps[:], rsbr[:, nsl], mult)
                        nc.gpsimd.tensor_tensor(O2[:, sl], CT[:, sl], AT[:, sl], mult)
                        if n == 1:
                            ksl = slice(m2 * LC, (m2 + 1) * LC)
                            nc.sync.dma_start(
                                out_d[b, 2 * P + m2 * P: 2 * P + (m2 + 1) * P, :],
                                AT[:, ksl],
                            )
                            nc.sync.dma_start(
                                out_d[b, 4 * P + m2 * P: 4 * P + (m2 + 1) * P, :],
                                O2[:, ksl],
                            )

                # ---- P3T = H2^T(as lhsT) @ E0T -> BT ; O3 = CT*BT ----
                BT = bt_pool.tile([P, KD * LC], f32)
                O3 = o3_pool.tile([P, KD * LC], f32, tag="e0tA")
                for m2 in range(KD):
                    for n in range(2):
                        ps = psB.tile([P, 512], f32, tag="psB")
                        for t in range(TQ):
                            nc.tensor.matmul(
                                ps[:],
                                r(H2[:, t * D + m2 * P: t * D + (m2 + 1) * P]),
                                r(E0T[:, t * LC + n * 512: t * LC + (n + 1) * 512]),
                                start=(t == 0),
                                stop=(t == TQ - 1),
                            )
                        sl = slice(m2 * LC + n * 512, m2 * LC + (n + 1) * 512)
                        nsl = slice(n * 512, (n + 1) * 512)
                        nc.vector.tensor_tensor(BT[:, sl], ps[:], rsbr[:, nsl], mult)
                        nc.gpsimd.tensor_tensor(O3[:, sl], CT[:, sl], BT[:, sl], mult)
                        if n == 1:
                            ksl = slice(m2 * LC, (m2 + 1) * LC)
                            nc.sync.dma_start(
                                out_d[b, 6 * P + m2 * P: 6 * P + (m2 + 1) * P, :],
                                O3[:, ksl],
                            )


    nc.compile()
    return nc


def _get_nc(mm_relaxed=MM_RELAXED):
    key = ("nc", mm_relaxed)
    if key not in _CACHE:
        _CACHE[key] = _build_nc(mm_relaxed)
    return _CACHE[key]


def kernel(C, Q, w4C, w4Q, w4mlu, bias=None, trace=False, **_ignored):
    _ensure_path()
    from concourse.bass_utils import run_bass_kernel_spmd

    C = np.ascontiguousarray(np.asarray(C, dtype=np.float32))
    Q = np.ascontiguousarray(np.asarray(Q, dtype=np.float32))
    w4C = np.ascontiguousarray(np.asarray(w4C, dtype=np.float32))
    w4Q = np.ascontiguousarray(np.asarray(w4Q, dtype=np.float32))
    w4mlu = np.ascontiguousarray(np.asarray(w4mlu, dtype=np.float32))

    nc = _get_nc()
    in_maps = []
    for i in range(NCORES):
        bsl = slice(i * BPC, (i + 1) * BPC)
        in_maps.append({
            "C": np.ascontiguousarray(C[:, bsl, :]),
            "Q": np.ascontiguousarray(Q[:, bsl, :]),
            "w4C": w4C,
            "w4Q": w4Q,
            "w4mlu": w4mlu,
        })
    res = run_bass_kernel_spmd(nc, in_maps, core_ids=list(range(NCORES)),
                               trace=trace)
    _CACHE["last_result"] = res
    outs = [res.results[i]["out"] for i in range(NCORES)]
    return np.concatenate(outs, axis=0)



# revision 10
# speedup vs baseline: 1.1521x; 1.1521x over previous
"""CQAttention (trilinear context-query attention) Bass kernel for TRN2.

Full-input contract: kernel(**inputs) takes the unsharded tensors
  C (1024, 64, 256), Q (512, 64, 256), w4C (256,1), w4Q (256,1),
  w4mlu (1,1,256), bias (1,)
and returns out (64, 1024, 1024) fp32, matching the reference

  C,Q -> batch-major; S = C@w4C + (Q@w4Q)^T + (C*w4mlu)@Q^T + bias
  S1 = softmax_q(S); S2 = softmax_c(S)
  A = S1@Q ; B = (S1@S2^T)@C
  out = concat([C, A, C*A, C*B], -1) transposed to (B, 4D, Lc)

Sharding: data-parallel over batch, 8 batch items per NeuronCore.

Algebra used on-chip (per batch item):
  * bias cancels in both softmaxes (constant shift) -> dropped.
  * e0 = exp(C@w4C - ln4), e1 = exp(Q@w4Q)
  * E0T = exp(s0^T - ln16) fp32   [q, c]   (scores, transposed layout)
  * E08 = fp8(4 * E0T^T)          [c, q]   (PE transpose of E0T)
  * rs[c] = sum_q e1[q] E0T[q,c]  (e1-broadcast lhsT matmul -> rsbr)
  * cs[q] = sum_c e0[c] E08[c,q]  (rides as appended e0 columns in Ce8)
  * A   = (Qe^T @ E0T) * (1/rs),      Qe  = Q * e1
  * H2e = (E08^T @ Ce8) * (e1/cs)     (= e1 * (S2^T C)[q,d])
  * B   = (H2e^T @ E0T) * (1/rs)
  The ln4/ln16 scales guard fp8e4m3 saturation and cancel exactly in the
  softmax normalizations.  Everything is computed transposed
  ([feature, context] layout) so output DMA rows are contiguous in DRAM.

P2 (the E08^T @ Ce8 contraction over Lc) runs in fp8e4m3 with the
DoubleRow perf mode (two 128-chunks of the contraction per matmul,
0.5 cyc/row); the A/B-path matmuls stay fp32r so the C*A channel keeps
full precision (single-fp8 weights there break the 2e-2 gate; measured
in emu.py).
"""

import numpy as np

LC, LQ, B, D = 1024, 512, 64, 256
NCORES = 8
BPC = B // NCORES  # batch items per core
P = 128
MC = LC // P  # 8 context chunks
TQ = LQ // P  # 4 query chunks
KD = D // P   # 2 feature chunks
DA = D + 2    # Ce payload + 2 appended e0 columns (cs rides along)
LN4 = 1.3862943611198906
LN16 = 2.772588722239781

_CACHE = {}


def _ensure_path():
    import sys
    for p in ("/opt/trn_rl_repo",):
        if p not in sys.path:
            sys.path.insert(0, p)


def _build_nc():
    _ensure_path()
    import concourse.bass as bass
    import concourse.bacc as bacc
    import concourse.mybir as mybir
    from concourse import tile, masks

    f32 = mybir.dt.float32
    f32r = mybir.dt.float32r
    f8 = mybir.dt.float8e4
    Exp = mybir.ActivationFunctionType.Exp
    Copy = mybir.ActivationFunctionType.Copy
    mult = mybir.AluOpType.mult
    add = mybir.AluOpType.add
    AxX = mybir.AxisListType.X
    DR = mybir.MatmulPerfMode.DoubleRow

    def r(ap):
        return ap.bitcast(f32r)

    nc = bacc.Bacc()
    C_d = nc.dram_tensor("C", [LC, BPC, D], f32, kind="ExternalInput")
    Q_d = nc.dram_tensor("Q", [LQ, BPC, D], f32, kind="ExternalInput")
    w4C_d = nc.dram_tensor("w4C", [D, 1], f32, kind="ExternalInput")
    w4Q_d = nc.dram_tensor("w4Q", [D, 1], f32, kind="ExternalInput")
    w4mlu_d = nc.dram_tensor("w4mlu", [1, 1, D], f32, kind="ExternalInput")
    out_d = nc.dram_tensor("out", [BPC, 4 * D, LC], f32, kind="ExternalOutput")

    with tile.TileContext(nc) as tc:
        import contextlib

        with contextlib.ExitStack() as ctx:
            ep = ctx.enter_context

            consts = ep(tc.tile_pool(name="consts", bufs=1))
            cn_pool = ep(tc.tile_pool(name="cn", bufs=2))
            qn_pool = ep(tc.tile_pool(name="qn", bufs=2))
            scr_pool = ep(tc.tile_pool(name="scr", bufs=2))
            ct_pool = ep(tc.tile_pool(name="ct", bufs=2))
            qmt_pool = ep(tc.tile_pool(name="qmt", bufs=2))
            qe_pool = ep(tc.tile_pool(name="qe", bufs=2))
            ce_pool = ep(tc.tile_pool(name="ce", bufs=2))
            e0_pool = ep(tc.tile_pool(name="e0p", bufs=2))
            e0t_pool = ep(tc.tile_pool(name="e0tp", bufs=2))
            h2_pool = ep(tc.tile_pool(name="h2", bufs=2))
            rsbr_pool = ep(tc.tile_pool(name="rsbr", bufs=2))
            e1br_pool = ep(tc.tile_pool(name="e1br", bufs=2))
            at_pool = ep(tc.tile_pool(name="at", bufs=2))
            bt_pool = ep(tc.tile_pool(name="bt", bufs=2))
            o2_pool = ep(tc.tile_pool(name="o2", bufs=2))
            o3_pool = ep(tc.tile_pool(name="o3", bufs=2))
            small_pool = ep(tc.tile_pool(name="small", bufs=4))

            psA = ep(tc.tile_pool(name="psA", bufs=4, space="PSUM"))
            psB = ep(tc.tile_pool(name="psB", bufs=4, space="PSUM"))

            # ---- per-core constants ----
            identf = consts.tile([P, P], f32)
            masks.make_identity(nc, identf[:])
            ident = consts.tile([P, P], f32r)
            nc.scalar.copy(ident[:], identf[:])
            negln4 = consts.tile([P, 1], f32)
            nc.vector.memset(negln4[:], -LN4)
            negln16 = consts.tile([P, 1], f32)
            nc.vector.memset(negln16[:], -LN16)
            c4 = consts.tile([P, 1], f32)
            nc.vector.memset(c4[:], 4.0)
            # w4mlu laid out [d-local partition, k chunk]
            w4mlu_pp = consts.tile([P, KD], f32)
            nc.sync.dma_start(
                w4mlu_pp[:], w4mlu_d[0, 0, :].rearrange("(k p) -> p k", p=P)
            )
            # w4C duplicated to column pairs (fp32r matmul needs even free)
            w4C2 = consts.tile([P, 2 * KD], f32)
            for j in range(2):
                nc.sync.dma_start(
                    r(w4C2[:].rearrange("p (k a) -> p k a", a=2)[:, :, j]),
                    r(w4C_d[:, 0].rearrange("(k p) -> p k", p=P)),
                )
            # w4Q replicated across partitions for the gpsimd pre-multiply
            w4Qb = consts.tile([P, D], f32)
            nc.sync.dma_start(
                w4Qb[:],
                w4Q_d[:, 0].rearrange("(a d) -> a d", a=1).broadcast_to([P, D]),
            )
            w4Qb_bc = w4Qb[:].rearrange("p (a d) -> p a d", a=1) \
                .broadcast_to([P, TQ, D])

            _pro_state = {}

            def _prologue(b):
                # batched loads (1 DMA each) + gpsimd pre-multiply for sub1
                Cn = cn_pool.tile([P, MC * D], f32, tag="cn")
                nc.sync.dma_start(
                    r(Cn[:]).rearrange("p (m d) -> p m d", m=MC),
                    r(C_d[:, b, :]).rearrange("(m p) d -> p m d", p=P),
                )
                Qn = qn_pool.tile([P, TQ * D], f32, tag="qn")
                nc.sync.dma_start(
                    r(Qn[:]).rearrange("p (t d) -> p t d", t=TQ),
                    r(Q_d[:, b, :]).rearrange("(t p) d -> p t d", p=P),
                )
                scr1 = scr_pool.tile([P, TQ * D], f32, tag="scr")
                nc.gpsimd.tensor_tensor(
                    scr1[:].rearrange("p (t d) -> p t d", t=TQ),
                    Qn[:].rearrange("p (t d) -> p t d", t=TQ),
                    w4Qb_bc, mult,
                )
                _pro_state[b] = (Cn, Qn, scr1)

            _prologue(0)
            for b in range(BPC):
                Cn, Qn, scr1 = _pro_state.pop(b)
                if b + 1 < BPC:
                    _prologue(b + 1)

                # ---- transposes: CT [d,(k,c)] (DVE evac), QmT [d,(k,q)]
                # (ACT evac fused with the w4mlu per-partition scale) ----
                CT = ct_pool.tile([P, KD * LC], f32)
                for k in range(KD):
                    for mg in range(0, MC, 4):
                        pst = psA.tile([P, 4 * P], f32, tag="psA")
                        for j in range(4):
                            m = mg + j
                            nc.tensor.transpose(
                                r(pst[:, j * P:(j + 1) * P]),
                                r(Cn[:, m * D + k * P: m * D + (k + 1) * P]),
                                ident[:],
                            )
                        nc.vector.tensor_copy(
                            r(CT[:, k * LC + mg * P: k * LC + (mg + 4) * P]),
                            pst[:],
                        )
                QmT = qmt_pool.tile([P, KD * LQ], f32)
                for k in range(KD):
                    pst = psA.tile([P, 4 * P], f32, tag="psA")
                    for t in range(TQ):
                        nc.tensor.transpose(
                            r(pst[:, t * P:(t + 1) * P]),
                            r(Qn[:, t * D + k * P: t * D + (k + 1) * P]),
                            ident[:],
                        )
                    nc.scalar.activation(
                        r(QmT[:, k * LQ:(k + 1) * LQ]), pst[:], Copy,
                        scale=w4mlu_pp[:, k:k + 1],
                    )

                # C passthrough block of the output
                nc.sync.dma_start(
                    out_d[b, 0:KD * P, :].rearrange("(k p) c -> p k c", p=P),
                    CT[:].rearrange("p (k c) -> p k c", k=KD),
                )

                # ---- sub0 = C@w4C via thin matmuls; e0 = exp(sub0 - ln4) ----
                ps0 = psB.tile([P, 2 * MC], f32, tag="psB")
                for m in range(MC):
                    for k in range(KD):
                        nc.tensor.matmul(
                            ps0[:, 2 * m:2 * m + 2],
                            r(CT[:, k * LC + m * P: k * LC + (m + 1) * P]),
                            r(w4C2[:, 2 * k:2 * k + 2]),
                            start=(k == 0),
                            stop=(k == KD - 1),
                        )
                e0 = small_pool.tile([P, 2 * MC], f32)
                nc.scalar.activation(e0[:], ps0[:], Exp, bias=negln4[:])

                # sub1 = Q@w4Q (free-dim reduce of gpsimd pre-multiply);
                # e1 = exp(sub1)
                sub1 = small_pool.tile([P, TQ], f32)
                nc.vector.tensor_reduce(
                    sub1[:], scr1[:].rearrange("p (t d) -> p t d", t=TQ),
                    axis=AxX, op=add,
                )
                e1 = small_pool.tile([P, TQ], f32)
                nc.scalar.activation(e1[:], sub1[:], Exp)

                # ---- Qe = Q * e1 (f32r, AT lhsT); Ce8 = [C*e0 | e0 e0] ----
                Qe = qe_pool.tile([P, TQ * D], f32)
                for t in range(TQ):
                    nc.vector.tensor_scalar_mul(
                        r(Qe[:, t * D:(t + 1) * D]), Qn[:, t * D:(t + 1) * D],
                        e1[:, t:t + 1],
                    )
                Ce8 = ce_pool.tile([P, MC * DA], f8)
                for m in range(MC):
                    nc.vector.tensor_scalar_mul(
                        Ce8[:, m * DA:m * DA + D], Cn[:, m * D:(m + 1) * D],
                        e0[:, 2 * m:2 * m + 1],
                    )
                    nc.vector.tensor_copy(
                        Ce8[:, m * DA + D:m * DA + DA],
                        e0[:, 2 * m:2 * m + 2],
                    )

                # ---- E0T = exp(s0^T - ln16) [q,(t,c)] fp32 ----
                E0T = e0t_pool.tile([P, TQ * LC], f32, tag="e0t")
                for t in range(TQ):
                    for n in range(2):
                        ps = psA.tile([P, 512], f32, tag="psA")
                        for k in range(KD):
                            nc.tensor.matmul(
                                ps[:],
                                r(QmT[:, k * LQ + t * P: k * LQ + (t + 1) * P]),
                                r(CT[:, k * LC + n * 512: k * LC + (n + 1) * 512]),
                                start=(k == 0),
                                stop=(k == KD - 1),
                            )
                        nc.scalar.activation(
                            r(E0T[:, t * LC + n * 512: t * LC + (n + 1) * 512]),
                            ps[:], Exp, bias=negln16[:],
                        )

                # e1 broadcast along free dim (rsbr lhsT)
                e1br = e1br_pool.tile([P, TQ * P], f32)
                for t in range(TQ):
                    nc.vector.tensor_copy(
                        r(e1br[:, t * P:(t + 1) * P]),
                        e1[:, t:t + 1].broadcast_to([P, P]),
                    )

                # ---- rsbr = 1 / (e1 row-weighted colsum of E0T) ----
                rsbr = rsbr_pool.tile([P, LC], f32, tag="rsbr")
                for n in range(2):
                    ps = psB.tile([P, 512], f32, tag="psB")
                    for t in range(TQ):
                        nc.tensor.matmul(
                            ps[:],
                            r(e1br[:, t * P:(t + 1) * P]),
                            r(E0T[:, t * LC + n * 512: t * LC + (n + 1) * 512]),
                            start=(t == 0),
                            stop=(t == TQ - 1),
                        )
                    nc.vector.reciprocal(rsbr[:, n * 512:(n + 1) * 512], ps[:])

                # ---- E08 = fp8(4 * E0T^T) [c,(m,q)] via PE transposes ----
                E08 = e0_pool.tile([P, MC * LQ], f8)
                for m in range(MC):
                    pst = psA.tile([P, LQ], f32, tag="psA")
                    for t in range(TQ):
                        nc.tensor.transpose(
                            r(pst[:, t * P:(t + 1) * P]),
                            r(E0T[:, t * LC + m * P: t * LC + (m + 1) * P]),
                            ident[:],
                        )
                    nc.scalar.activation(
                        E08[:, m * LQ:(m + 1) * LQ], pst[:], Copy,
                        scale=c4[:],
                    )

                # ---- P2 = E08^T @ Ce8 (fp8 DoubleRow over m-pairs);
                #      H2e = P2 * (e1/cs) ----
                E08v = E08[:].rearrange("p (m q) -> p m q", m=MC)
                Ce8v = Ce8[:].rearrange("p (m e) -> p m e", m=MC)
                rec_cse = small_pool.tile([P, TQ], f32)
                dq = small_pool.tile([P, TQ], f32)
                H2e = h2_pool.tile([P, TQ * D], f32)
                for qm in range(TQ):
                    ps = psB.tile([P, 512], f32, tag="psB")
                    for mp in range(MC // 2):
                        nc.tensor.matmul(
                            ps[:, 0:DA],
                            E08v[:, 2 * mp:2 * mp + 2, qm * P:(qm + 1) * P],
                            Ce8v[:, 2 * mp:2 * mp + 2, :],
                            start=(mp == 0),
                            stop=(mp == MC // 2 - 1),
                            perf_mode=DR,
                        )
                    nc.vector.reciprocal(rec_cse[:, qm:qm + 1], ps[:, D:D + 1])
                    nc.vector.tensor_tensor(
                        dq[:, qm:qm + 1], rec_cse[:, qm:qm + 1],
                        e1[:, qm:qm + 1], mult,
                    )
                    nc.scalar.activation(
                        r(H2e[:, qm * D:(qm + 1) * D]), ps[:, 0:D], Copy,
                        scale=dq[:, qm:qm + 1],
                    )

                # ---- AT = (Qe^T @ E0T) * rsbr ; O2 = CT*AT ----
                AT = at_pool.tile([P, KD * LC], f32)
                O2 = o2_pool.tile([P, KD * LC], f32)
                for m2 in range(KD):
                    for n in range(2):
                        ps = psB.tile([P, 512], f32, tag="psB")
                        for t in range(TQ):
                            nc.tensor.matmul(
                                ps[:],
                                r(Qe[:, t * D + m2 * P: t * D + (m2 + 1) * P]),
                                r(E0T[:, t * LC + n * 512: t * LC + (n + 1) * 512]),
                                start=(t == 0),
                                stop=(t == TQ - 1),
                            )
                        sl = slice(m2 * LC + n * 512, m2 * LC + (n + 1) * 512)
                        nsl = slice(n * 512, (n + 1) * 512)
                        nc.vector.tensor_tensor(AT[:, sl], ps[:], rsbr[:, nsl], mult)
                        nc.gpsimd.tensor_tensor(O2[:, sl], CT[:, sl], AT[:, sl], mult)
                nc.sync.dma_start(
                    out_d[b, 2 * P:4 * P, :].rearrange("(k p) c -> p k c", p=P),
                    AT[:].rearrange("p (k c) -> p k c", k=KD),
                )
                nc.sync.dma_start(
                    out_d[b, 4 * P:6 * P, :].rearrange("(k p) c -> p k c", p=P),
                    O2[:].rearrange("p (k c) -> p k c", k=KD),
                )

                # ---- BT = (H2e^T @ E0T) * rsbr ; O3 = CT*BT ----
                BT = bt_pool.tile([P, KD * LC], f32)
                O3 = o3_pool.tile([P, KD * LC], f32)
                for m2 in range(KD):
                    for n in range(2):
                        ps = psB.tile([P, 512], f32, tag="psB")
                        for t in range(TQ):
                            nc.tensor.matmul(
                                ps[:],
                                r(H2e[:, t * D + m2 * P: t * D + (m2 + 1) * P]),
                                r(E0T[:, t * LC + n * 512: t * LC + (n + 1) * 512]),
                                start=(t == 0),
                                stop=(t == TQ - 1),
                            )
                        sl = slice(m2 * LC + n * 512, m2 * LC + (n + 1) * 512)
                        nsl = slice(n * 512, (n + 1) * 512)
                        nc.vector.tensor_tensor(BT[:, sl], ps[:], rsbr[:, nsl], mult)
                        nc.gpsimd.tensor_tensor(O3[:, sl], CT[:, sl], BT[:, sl], mult)
                nc.sync.dma_start(
                    out_d[b, 6 * P:8 * P, :].rearrange("(k p) c -> p k c", p=P),
                    O3[:].rearrange("p (k c) -> p k c", k=KD),
                )

    nc.compile()
    return nc


def _get_nc():
    key = "nc"
    if key not in _CACHE:
        _CACHE[key] = _build_nc()
    return _CACHE[key]


def kernel(C, Q, w4C, w4Q, w4mlu, bias=None, trace=False, **_ignored):
    _ensure_path()
    from concourse.bass_utils import run_bass_kernel_spmd

    C = np.ascontiguousarray(np.asarray(C, dtype=np.float32))
    Q = np.ascontiguousarray(np.asarray(Q, dtype=np.float32))
    w4C = np.ascontiguousarray(np.asarray(w4C, dtype=np.float32))
    w4Q = np.ascontiguousarray(np.asarray(w4Q, dtype=np.float32))
    w4mlu = np.ascontiguousarray(np.asarray(w4mlu, dtype=np.float32))

    nc = _get_nc()
    in_maps = []
    for i in range(NCORES):
        bsl = slice(i * BPC, (i + 1) * BPC)
        in_maps.append({
            "C": np.ascontiguousarray(C[:, bsl, :]),
            "Q": np.ascontiguousarray(Q[:, bsl, :]),
            "w4C": w4C,
            "w4Q": w4Q,
            "w4mlu": w4mlu,
        })
    res = run_bass_kernel_spmd(nc, in_maps, core_ids=list(range(NCORES)),
                               trace=trace)
    _CACHE["last_result"] = res
    outs = [res.results[i]["out"] for i in range(NCORES)]
    return np.concatenate(outs, axis=0)
